# revision 2
# baseline (speedup 1.0000x reference)
"""TransE edge scoring v3: PSUM-fused pipeline on 8 NeuronCores.

out[e] = sum_d | h[row[e], d] + g[type[e], d] - h[col[e], d] |

Per 128-edge block, everything accumulates in PSUM via the otherwise-idle
PE: an fp8 selector matmul adds g[type] (staged 64 rows per chunk), and
two identity matmuls add the dma_gather'd h[row] and -h[col] (negated
col table). One DVE tensor_reduce(|.|) per half-chunk reads PSUM
directly. No DVE subtract chain, no Act, no g-row gathers.

Per-core cost (cost-model ns/edge): DMA 3.2 (12 x 1024-row gathers per
6144-edge granule + selector/idx loads), Pool 2.95 (SWDGE desc-gen +
staging), PE ~1.3 at full clock, DVE ~1.4. DMA-bound.

Sharding: core = row-half x col-half quadrant (2 cores per quadrant,
split evenly). Edges type-sorted per core; chunks of 3072 edges with
<=64 distinct rel types (guaranteed by uniform types; checked).
"""

import sys

sys.path.insert(0, "/opt/trn_rl_repo")

import numpy as np
import ml_dtypes

import concourse.tile as tile
from concourse import bacc, mybir
from concourse.bass_utils import run_bass_kernel_spmd

N_NODES = 50000
N_REL = 500
D = 128
N_EDGES = 600000
NCORES = 8

HALF = 25000
CHUNK = 3072
EB = CHUNK // 128     # 24 edge-blocks per chunk
GR = 2 * CHUNK
G1024 = 1024
SLOTS = 64            # rel-type slots per chunk

F8 = ml_dtypes.float8_e4m3fn

_programs: dict[int, "bacc.Bacc"] = {}


def _wrap16(ids: np.ndarray) -> np.ndarray:
    """[n] -> [128, n//16] int16: idx i at [i%16, i//16], replicated x8."""
    n = len(ids)
    w = ids.reshape(n // 16, 16).T.astype(np.int16)
    return np.ascontiguousarray(np.tile(w, (8, 1)))


def _build_program(nch: int) -> "bacc.Bacc":
    ngr = -(-nch // 2)
    nc = bacc.Bacc("TRN2", debug=False, dynamic_dma_scratch_size=32768)
    dt = mybir.dt

    rows16 = nc.declare_dram_parameter("rows16", [HALF, D], dt.float16,
                                       isOutput=False)
    cols16n = nc.declare_dram_parameter("cols16n", [HALF, D], dt.float16,
                                        isOutput=False)
    gtab = nc.declare_dram_parameter("gtab", [N_REL, D], dt.float16,
                                     isOutput=False)
    ident = nc.declare_dram_parameter("ident", [128, 128], dt.float16,
                                      isOutput=False)
    # per granule: [0:384] row idx (6144), [384:576] col idx chunk0,
    # [576:768] col idx chunk1
    idx = nc.declare_dram_parameter("idx", [ngr, 128, 768], dt.int16,
                                    isOutput=False)
    gidx = nc.declare_dram_parameter("gidx", [nch, 128, 4], dt.int16,
                                     isOutput=False)
    sel = nc.declare_dram_parameter("sel", [nch, SLOTS, CHUNK], dt.float8e4,
                                    isOutput=False)
    scores = nc.declare_dram_parameter("scores", [nch, 128, EB], dt.float32,
                                       isOutput=True)

    with tile.TileContext(nc) as tc:
        with tc.tile_pool(name="const", bufs=1) as cp, \
             tc.tile_pool(name="idxp", bufs=3) as ip, \
             tc.tile_pool(name="gip", bufs=4) as gip, \
             tc.tile_pool(name="selp", bufs=4) as sp, \
             tc.tile_pool(name="hrp", bufs=2) as hrp, \
             tc.tile_pool(name="hcp", bufs=3) as hcp, \
             tc.tile_pool(name="stgp", bufs=4) as stp, \
             tc.tile_pool(name="scp", bufs=3) as scp, \
             tc.tile_pool(name="psm", bufs=2, space="PSUM") as psm:
            idt = cp.tile([128, 128], dt.float16, tag="idt")
            nc.sync.dma_start(idt[:], ident[:])

            for g in range(ngr):
                ncc = min(2, nch - 2 * g)       # chunks in this granule
                it = ip.tile([128, 768], dt.int16, tag="it")
                nc.sync.dma_start(it[:], idx[g])

                hr = hrp.tile([128, GR // 128, D], dt.float16, tag="hr")
                for j in range(3 * ncc):
                    nc.gpsimd.dma_gather(
                        hr[:, 8 * j:8 * (j + 1), :], rows16[:],
                        it[:, 64 * j:64 * (j + 1)],
                        num_idxs=G1024, num_idxs_reg=G1024, elem_size=D)

                for c in range(ncc):
                    k = 2 * g + c
                    iofs = 384 + 192 * c
                    git = gip.tile([128, 4], dt.int16, tag="git")
                    nc.sync.dma_start(git[:], gidx[k])
                    sl = sp.tile([SLOTS, CHUNK], dt.float8e4, tag="sl")
                    nc.sync.dma_start(sl[:], sel[k])
                    stg = stp.tile([128, 1, D], dt.float16, tag="stg")
                    nc.gpsimd.dma_gather(stg[:], gtab[:], git[:],
                                         num_idxs=SLOTS, num_idxs_reg=SLOTS,
                                         elem_size=D)
                    hc = hcp.tile([128, EB, D], dt.float16, tag="hc")
                    for j in range(3):
                        nc.gpsimd.dma_gather(
                            hc[:, 8 * j:8 * (j + 1), :], cols16n[:],
                            it[:, iofs + 64 * j:iofs + 64 * (j + 1)],
                            num_idxs=G1024, num_idxs_reg=G1024, elem_size=D)

                    sct = scp.tile([128, EB], dt.float32, tag="sct")
                    for h in range(2):
                        ps = psm.tile([128, EB // 2, D], dt.float32, tag="ps")
                        for b in range(EB // 2):
                            eb = (EB // 2) * h + b
                            po = ps[:, b, :]
                            nc.tensor.matmul(
                                po, sl[:, 128 * eb:128 * (eb + 1)],
                                stg[0:SLOTS, 0, :], start=True, stop=False)
                            nc.tensor.matmul(
                                po, idt[:], hr[:, EB * c + eb, :],
                                start=False, stop=False)
                            nc.tensor.matmul(
                                po, idt[:], hc[:, eb, :],
                                start=False, stop=True)
                        nc.vector.tensor_reduce(
                            sct[:, (EB // 2) * h:(EB // 2) * (h + 1)],
                            ps[:], axis=mybir.AxisListType.X,
                            op=mybir.AluOpType.add,
                            apply_absolute_value=True)
                    nc.sync.dma_start(scores[k], sct[:])
    nc.compile()
    return nc


def kernel(h, g, edge_idx, edge_type):
    h = np.asarray(h, dtype=np.float32)
    g = np.asarray(g, dtype=np.float32)
    row = np.asarray(edge_idx[0]).astype(np.int64)
    col = np.asarray(edge_idx[1]).astype(np.int64)
    typ = np.asarray(edge_type).astype(np.int64)

    h16 = h.astype(np.float16)
    g16 = np.ascontiguousarray(g.astype(np.float16))
    half_tabs = [np.ascontiguousarray(h16[:HALF]),
                 np.ascontiguousarray(h16[HALF:])]
    halfn_tabs = [np.ascontiguousarray(-h16[:HALF]),
                  np.ascontiguousarray(-h16[HALF:])]
    ident = np.ascontiguousarray(np.eye(128, dtype=np.float16))

    # quadrant -> 2 cores, split edges evenly
    quad = (row // HALF) * 2 + (col // HALF)
    order_q = np.argsort(quad, kind="stable")
    qcounts = np.bincount(quad, minlength=4)
    core_edges = []
    off = 0
    for q in range(4):
        ids = order_q[off:off + qcounts[q]]
        off += qcounts[q]
        halfn = (len(ids) + 1) // 2
        core_edges.append(ids[:halfn])
        core_edges.append(ids[halfn:])

    nch = max(1, -(-max(len(e) for e in core_edges) // CHUNK))
    ngr = -(-nch // 2)

    if nch not in _programs:
        _programs[nch] = _build_program(nch)
    nc = _programs[nch]

    in_maps = []
    placements = []
    for ci in range(8):
        ids = core_edges[ci]
        q = ci // 2
        rh, ch_ = q // 2, q % 2
        o = np.argsort(typ[ids], kind="stable")
        ids = ids[o]
        n = len(ids)
        place = np.full(nch * CHUNK, -1, np.int64)
        place[:n] = ids
        rl = np.zeros(nch * CHUNK, np.int64)
        cl = np.zeros(nch * CHUNK, np.int64)
        ty = np.zeros(nch * CHUNK, np.int64)
        rl[:n] = row[ids] - HALF * rh
        cl[:n] = col[ids] - HALF * ch_
        ty[:n] = typ[ids]
        placements.append(place)

        sel_arr = np.zeros((nch, SLOTS, CHUNK), np.float16)
        gidx_arr = np.zeros((nch, SLOTS), np.int64)
        idx_arr = np.zeros((ngr, 128, 768), np.int16)
        gidx16 = np.zeros((nch, 128, 4), np.int16)
        for k in range(nch):
            ce = slice(k * CHUNK, (k + 1) * CHUNK)
            uniq, inv = np.unique(ty[ce], return_inverse=True)
            if len(uniq) > SLOTS:
                raise RuntimeError(f"chunk {k}: {len(uniq)} types > {SLOTS}")
            gidx_arr[k, :len(uniq)] = uniq
            valid = place[ce] >= 0
            e = np.arange(CHUNK)
            sel_arr[k, inv[valid], e[valid]] = 1.0
            gi, c = k // 2, k % 2
            iofs = 384 + 192 * c
            idx_arr[gi, :, iofs:iofs + 192] = _wrap16(cl[ce])
            gidx16[k] = _wrap16(gidx_arr[k])
        for gi in range(ngr):
            go = gi * GR
            idx_arr[gi, :, 0:384] = _wrap16(
                np.concatenate([rl[go:go + GR],
                                np.zeros(max(0, go + GR - nch * CHUNK),
                                         np.int64)])[:GR])

        in_maps.append({
            "rows16": half_tabs[rh],
            "cols16n": halfn_tabs[ch_],
            "gtab": g16,
            "ident": ident,
            "idx": idx_arr,
            "gidx": gidx16,
            "sel": sel_arr.astype(F8),
            "scores": np.zeros((nch, 128, EB), np.float32),
        })

    results = run_bass_kernel_spmd(nc, in_maps, list(range(NCORES))).results

    out = np.empty(N_EDGES, dtype=np.float32)
    for ci in range(NCORES):
        sc = np.asarray(results[ci]["scores"])        # [nch, 128, EB]
        vals = sc.transpose(0, 2, 1).reshape(-1)      # edge = 128*eb + p
        place = placements[ci]
        m = place >= 0
        out[place[m]] = vals[m]
    return out
